# revision 1
# baseline (speedup 1.0000x reference)
"""Bass/Tile kernel builder for the bidirectional LSTM (S=512, B=64, I=H=512).

Sharding: 8 cores, each runs BOTH directions on a batch slice of 8.
Per core:
  Phase 1: xproj[d] = x[d] @ W_ih[d].T + b  (big GEMM, weights-stationary)
           -> DRAM ring, layout [d, tok_tile, chunk, 128, 512]
  Phase 2: 512-step recurrence.
    Gate layout (per direction d, col groups gA/gB):
      group holds all four gates' column-chunk: [i c | f c | o c | g c] (256 each)
      fwd: groups 0 (cols 0:256) and 1 (cols 256:512) -> PSUM partitions 0-7, 32-39
      bwd: groups 2, 3 -> partitions 64-71, 96-103
    matmuls: lhsT = hT tile [128, 8] (h transposed, fp32r), rhs = W_hh
      reordered slab [128, 512], col-tiled via out base partition 32g.
    xproj injected via selector matmul (I8 stationary, rhs = xs slab [8,512]).
    ACT: tanh(g), sigmoid(i,f,o) from PSUM; DVE: c/h updates; PE transposes
    h chunks back into hT for the next step.
"""

import sys
if "/opt/trn_rl_repo" not in sys.path:
    sys.path.insert(0, "/opt/trn_rl_repo")
import numpy as np

import concourse.bass as bass
import concourse.bacc as bacc
import concourse.mybir as mybir
import concourse.tile as tile

F32 = mybir.dt.float32
F32R = mybir.dt.float32r
AF = mybir.ActivationFunctionType
ALU = mybir.AluOpType

S, B, I, H = 512, 64, 512, 512
NC = 8
BC = B // NC          # batch per core = 8
G4 = 4 * H            # 2048
CH = H // 2           # 256: gate column chunk per group
TOK_TILE = 128        # phase-1 token tile
N_TOK = S * BC        # 4096 tokens per direction per core
N_TT = N_TOK // TOK_TILE   # 32 token tiles
N_GC = G4 // 512      # 4 gate chunks of 512 in phase-1


def reorder_cols(dirn_mats):
    """Build the reordered gate-column permutation.

    W_hh rows are [i(512) | f(512) | g(512) | o(512)].  We want rhs columns
    ordered per group: group0 = [i0 f0 o0 g0] (chunks cols 0:256 of each
    gate), group1 = [i1 f1 o1 g1].
    Returns an index array perm[2048] such that reordered[:, j] = orig[:, perm[j]].
    """
    idx = []
    for gate in (2, 0, 1, 3):  # g, i, f, o
        idx.extend(range(gate * H, (gate + 1) * H))
    return np.array(idx, dtype=np.int64)


PERM = reorder_cols(None)


def prep_core_inputs(inpt, W_ih_f, W_hh_f, b_ih_f, b_hh_f,
                     W_ih_b, W_hh_b, b_ih_b, b_hh_b):
    """Host-side prep.  Returns (shared dict, per-core list of dicts)."""
    x_f = np.ascontiguousarray(inpt)          # [S, B, I]
    x_b = np.ascontiguousarray(inpt[::-1])    # flipped for backward scan

    shared = {}
    for d, (Wih, Whh, bih, bhh) in (("f", (W_ih_f, W_hh_f, b_ih_f, b_hh_f)),
                                    ("b", (W_ih_b, W_hh_b, b_ih_b, b_hh_b))):
        Wr_ih = np.ascontiguousarray(Wih.T[:, PERM], dtype=np.float32)  # [512,2048]
        Wr_hh = np.ascontiguousarray(Whh.T[:, PERM], dtype=np.float32)  # [512,2048]
        bias = np.ascontiguousarray((bih + bhh)[PERM], dtype=np.float32)[None, :]
        # SBUF slab layout [128, 4, 2048]: partition p, ktile k -> row 128k+p
        shared[f"Wih_{d}"] = np.ascontiguousarray(
            Wr_ih.reshape(4, 128, G4).transpose(1, 0, 2))
        shared[f"Whh_{d}"] = np.ascontiguousarray(
            Wr_hh.reshape(4, 128, G4).transpose(1, 0, 2))
        shared[f"bias_{d}"] = bias                                     # [1, 2048]

    sel_blk = np.zeros((32, 32), dtype=np.float32)
    sel_blk[0:8, 0:8] = np.eye(8, dtype=np.float32)
    shared["sel8"] = np.tile(sel_blk, (4, 1))                          # [128, 32]
    shared["ones1"] = np.ones((1, 128), dtype=np.float32)              # [1, 128]
    shared["ident"] = np.tile(np.eye(8, dtype=np.float32), (16, 1))    # [128, 8]

    in_maps = []
    for c in range(NC):
        bs = slice(c * BC, (c + 1) * BC)
        m = dict(shared)
        for d, x in (("f", x_f), ("b", x_b)):
            xs = x[:, bs, :]                       # [S, 8, I]
            xT = xs.reshape(S * BC, I).T           # [I, S*8] tokens t-major
            m[f"xT_{d}"] = np.ascontiguousarray(xT, dtype=np.float32)
        in_maps.append(m)
    return in_maps


def assemble_output(results):
    """results: list of 8 per-core dicts with out_f/out_b [S, 128, 4, 8]."""
    out = np.empty((S, B, 2 * H), dtype=np.float32)
    for c in range(NC):
        bs = slice(c * BC, (c + 1) * BC)
        for d, off in (("f", 0), ("b", H)):
            slab = results[c][f"out_{d}"]          # [S, 128, 4, 8] = [t, r, k, b]
            # h[t, b, 128k + r] = slab[t, r, k, b]
            h = slab.transpose(0, 3, 2, 1).reshape(S, BC, H)
            out[:, bs, off:off + H] = h.astype(np.float32)
    return out


def build_nc(n_steps=S, interleave=True):
    """Build the full Bass program. Returns nc."""
    nc = bacc.Bacc("TRN2", target_bir_lowering=False, debug=False)

    # ---- DRAM I/O -------------------------------------------------------
    dram = {}
    for d in ("f", "b"):
        dram[f"xT_{d}"] = nc.declare_dram_parameter(
            f"xT_{d}", [I, N_TOK], F32R, isOutput=False)
        dram[f"Wih_{d}"] = nc.declare_dram_parameter(
            f"Wih_{d}", [128, 4, G4], F32R, isOutput=False)
        dram[f"Whh_{d}"] = nc.declare_dram_parameter(
            f"Whh_{d}", [128, 4, G4], F32R, isOutput=False)
        dram[f"bias_{d}"] = nc.declare_dram_parameter(
            f"bias_{d}", [1, G4], F32R, isOutput=False)
        dram[f"out_{d}"] = nc.declare_dram_parameter(
            f"out_{d}", [n_steps, 128, 4, BC], F32R, isOutput=True)
    dram["sel8"] = nc.declare_dram_parameter("sel8", [128, 32], F32R, isOutput=False)
    dram["ones1"] = nc.declare_dram_parameter("ones1", [1, 128], F32R, isOutput=False)
    dram["ident"] = nc.declare_dram_parameter("ident", [128, 8], F32R, isOutput=False)

    # internal xproj ring in DRAM: [d, tok_tile, chunk, 128, 512]
    n_tt = (n_steps * BC + TOK_TILE - 1) // TOK_TILE
    xproj = {d: nc.dram_tensor(f"xproj_{d}", [n_tt, N_GC, TOK_TILE, 512], F32R)
             for d in ("f", "b")}

    DIRS = ("f", "b")
    # partition bases of the 4 col groups: fwd groups 0,1; bwd groups 2,3
    GRP = {"f": (0, 32), "b": (64, 96)}

    with tile.TileContext(nc) as tc:
        with (
            tc.tile_pool(name="weights", bufs=1) as wpool,
            tc.tile_pool(name="consts", bufs=1) as cpool,
            tc.tile_pool(name="p1w", bufs=1) as p1w,
            tc.tile_pool(name="p1x", bufs=2) as p1x,
            tc.tile_pool(name="p1out", bufs=2) as p1out,
            tc.tile_pool(name="p1ps", bufs=2, space="PSUM") as p1ps,
            tc.tile_pool(name="state", bufs=1) as spool,
            tc.tile_pool(name="xs", bufs=2) as xspool,
            tc.tile_pool(name="gps", bufs=2, space="PSUM") as gpspool,
            tc.tile_pool(name="tps", bufs=2, space="PSUM") as tpspool,
            tc.tile_pool(name="eltw", bufs=1) as epool,
        ):
            # ---- resident constants/weights --------------------------------
            Whh_sb = {}
            for d in DIRS:
                Whh_sb[d] = wpool.tile([128, 4, G4], F32R, tag=f"whh{d}", name=f"whh{d}")
                for k in range(4):
                    nc.sync.dma_start(Whh_sb[d][:, k, :], dram[f"Whh_{d}"][:, k, :])
            sel8 = cpool.tile([128, 32], F32R, tag="sel8")
            ones1 = cpool.tile([1, 128], F32R, tag="ones1")
            ident = cpool.tile([128, 8], F32R, tag="ident")
            nc.sync.dma_start(sel8[:, :], dram["sel8"][:, :])
            nc.sync.dma_start(ones1[:, :], dram["ones1"][:, :])
            nc.sync.dma_start(ident[:, :], dram["ident"][:, :])

            # ---- phase 1: xproj = xT.T @ Wih + bias ------------------------
            if True:
                Wih_sb, bias_sb = {}, {}
                for d in DIRS:
                    Wih_sb[d] = p1w.tile([128, 4, G4], F32R, tag=f"wih{d}",
                                         name=f"wih{d}")
                    bias_sb[d] = p1w.tile([1, G4], F32R, tag=f"bias{d}",
                                          name=f"biassb{d}")
                    for k in range(4):
                        nc.sync.dma_start(Wih_sb[d][:, k, :],
                                          dram[f"Wih_{d}"][:, k, :])
                    nc.sync.dma_start(bias_sb[d][:, :], dram[f"bias_{d}"][:, :])
                def emit_p1_tile(d, i):
                    xTd = dram[f"xT_{d}"].rearrange("(k p) t -> p k t", p=128)
                    xt = p1x.tile([128, 4, TOK_TILE], F32R, tag="xt", name=f"xt{d}{i}")
                    nc.sync.dma_start(
                        xt[:, :, :],
                        xTd[:, :, i * TOK_TILE:(i + 1) * TOK_TILE])
                    for c in range(N_GC):
                        ps = p1ps.tile([128, 512], F32, tag="p1ps", name=f"p1ps{d}{i}{c}")
                        for k in range(4):
                            nc.tensor.matmul(
                                ps[:, :],
                                xt[:, k, :],
                                Wih_sb[d][:, k, c * 512:(c + 1) * 512],
                                start=(k == 0), stop=False)
                        nc.tensor.matmul(
                            ps[:, :], ones1[:, :],
                            bias_sb[d][:, c * 512:(c + 1) * 512],
                            start=False, stop=True)
                        xo = p1out.tile([128, 512], F32R, tag="p1o", name=f"p1o{d}{i}{c}")
                        nc.scalar.copy(xo[:, :], ps[:, :])
                        nc.sync.dma_start(xproj[d][i, c, :, :], xo[:, :])

                P1_LOOK = 2
                for i in range(min(P1_LOOK, n_tt)):
                    for d in DIRS:
                        emit_p1_tile(d, i)

            # ---- phase 2: recurrence --------------------------------------
            # No matmul column tiling (walrus limitation): every matmul's
            # output sits at PSUM partitions 0..M.  Per (dir, half) the gates
            # accumulate in their own [32, 1024] PSUM tile; halves of the
            # reordered gate columns: half0 = [g | i], half1 = [f | o].
            hT = {d: [spool.tile([128, 4 * BC], F32R, tag=f"hT{d}{j}", name=f"hT{d}{j}")
                      for j in range(2)] for d in DIRS}
            cst = {d: [spool.tile([BC, H], F32, tag=f"c{d}{j}", name=f"cst{d}{j}")
                       for j in range(2)] for d in DIRS}
            for d in DIRS:
                nc.vector.memset(hT[d][0][:, :].bitcast(F32), 0.0)
                nc.vector.memset(cst[d][0][:, :], 0.0)

            for t in range(n_steps):
                cur, nxt = t % 2, (t + 1) % 2
                if t % 16 == 0:
                    nxt_tile = t // 16 + P1_LOOK
                    if nxt_tile < n_tt:
                        for d in DIRS:
                            emit_p1_tile(d, nxt_tile)
                # xs slabs: [128, 512] per dir; chunk c at partitions 32c
                xs = {}
                for d in DIRS:
                    xs[d] = xspool.tile([128, 512], F32R, tag=f"xs{d}",
                                        name=f"xs{d}")
                    tt, tr = (t * BC) // TOK_TILE, (t * BC) % TOK_TILE
                    for c in range(4):
                        nc.sync.dma_start(
                            xs[d][32 * c:32 * c + BC, :],
                            xproj[d][tt, c, tr:tr + BC, :])

                gh = {}
                for d in DIRS:
                    for half in range(2):
                        g = gpspool.tile([32, 1024], F32, tag="gh",
                                         name=f"gh{d}{half}")
                        gh[(d, half)] = g
                        for q in range(2):        # two 512-col quarters
                            c = half * 2 + q
                            o32 = g[0:32, q * 512:(q + 1) * 512]
                            nc.tensor.matmul(
                                o32, sel8[32 * c:32 * c + BC, :],
                                xs[d][32 * c:32 * c + BC, :],
                                start=True, stop=False,
                                skip_group_check=True,
                                tile_position=(32 * c, 0))
                            o = g[0:BC, q * 512:(q + 1) * 512]
                            for k in range(4):
                                nc.tensor.matmul(
                                    o, hT[d][cur][:, k * BC:(k + 1) * BC],
                                    Whh_sb[d][:, k, c * 512:(c + 1) * 512],
                                    start=False, stop=(k == 3),
                                    skip_group_check=True)

                for d in DIRS:
                    g0, g1 = gh[(d, 0)], gh[(d, 1)]
                    tg = epool.tile([BC, H], F32, tag=f"tg{d}", name=f"tg{d}")
                    si = epool.tile([BC, H], F32, tag=f"si{d}", name=f"si{d}")
                    sf = epool.tile([BC, H], F32, tag=f"sf{d}", name=f"sf{d}")
                    so = epool.tile([BC, H], F32, tag=f"so{d}", name=f"so{d}")
                    nc.scalar.activation(tg[:, :], g0[0:BC, 0:512], AF.Tanh)
                    nc.scalar.activation(si[:, :], g0[0:BC, 512:1024], AF.Sigmoid)
                    nc.scalar.activation(sf[:, :], g1[0:BC, 0:512], AF.Sigmoid)
                    nc.scalar.activation(so[:, :], g1[0:BC, 512:1024], AF.Sigmoid)

                    ig = epool.tile([BC, H], F32, tag=f"ig{d}", name=f"ig{d}")
                    fc = epool.tile([BC, H], F32, tag=f"fc{d}", name=f"fc{d}")
                    nc.vector.tensor_mul(ig[:, :], si[:, :], tg[:, :])
                    nc.vector.tensor_mul(fc[:, :], sf[:, :], cst[d][cur][:, :])
                    nc.vector.tensor_add(cst[d][nxt][:, :], ig[:, :], fc[:, :])
                    tc_t = epool.tile([BC, H], F32, tag=f"tc{d}", name=f"tc{d}")
                    nc.scalar.activation(tc_t[:, :], cst[d][nxt][:, :], AF.Tanh)
                    # staggered tail: per 128-col chunk k, the h multiply,
                    # transpose, and hT copy land independently so the next
                    # step's Ktile-k matmul unblocks as soon as chunk k is in.
                    ht = epool.tile([BC, H], F32R, tag=f"ht{d}", name=f"ht{d}")
                    pt = tpspool.tile([128, 4 * BC], F32R, tag="pt",
                                      name=f"pt{d}")
                    for k in range(4):
                        nc.vector.tensor_mul(ht[:, k * 128:(k + 1) * 128],
                                             so[:, k * 128:(k + 1) * 128],
                                             tc_t[:, k * 128:(k + 1) * 128])
                        nc.tensor.matmul(
                            pt[:, k * BC:(k + 1) * BC],
                            ht[:, k * 128:(k + 1) * 128],
                            ident[0:BC, :],
                            start=(k == 0), stop=(k == 3),
                            is_transpose=True,
                            skip_group_check=True)
                        nc.vector.tensor_copy(
                            hT[d][nxt][:, k * BC:(k + 1) * BC],
                            pt[:, k * BC:(k + 1) * BC])
                    nc.sync.dma_start(
                        dram[f"out_{d}"][t, :, :, :],
                        hT[d][nxt][:, :].rearrange("p (k b) -> p k b", k=4))

    nc.compile()
    return nc

# ---------------------------------------------------------------------------
# Entry point: kernel(**inputs) -> np.ndarray  [S, B, 2H]
# ---------------------------------------------------------------------------
from concourse.bass_utils import run_bass_kernel_spmd

_NC_CACHE = {}


def _get_nc():
    if "nc" not in _NC_CACHE:
        _NC_CACHE["nc"] = build_nc(n_steps=S)
    return _NC_CACHE["nc"]


def kernel(**inputs):
    nc = _get_nc()
    in_maps = prep_core_inputs(**inputs)
    res = run_bass_kernel_spmd(nc, in_maps, list(range(NC)))
    return assemble_output(res.results)



# revision 3
# speedup vs baseline: 11.2334x; 11.2334x over previous
"""Bass/Tile kernel for the bidirectional LSTM (S=512, B=64, I=H=512).

Sharding: direction-split + batch-split. Cores 0-3 run the FORWARD scan on
batch quarters of 16; cores 4-7 run the BACKWARD scan (input pre-flipped on
the host) on the same batch quarters. All cores execute the same SPMD
program; the direction lives entirely in the per-core weights/inputs.

Per core:
  Phase 1 (interleaved): xproj = x @ W_ih.T + b as a weights-stationary GEMM
    over 64 token tiles of 128 tokens, written to a DRAM ring
    xproj[tile, 128, 4, 512] (gate-chunk-major columns, PERM-reordered).
  Phase 2: 512-step recurrence, 8 steps per token tile.
    Gate columns reordered to chunks [g | i | f | o] (512 each).
    Per step: 4 selector matmuls inject xproj rows into the 4 gate PSUM
    banks (K=32 token window, eye(16) selector picks the step's 16 rows),
    then 4x4 W_hh matmuls (lhsT = h^T slab [128,16] per ktile) accumulate
    h @ W_hh. ACT applies tanh/sigmoid (f/o/tanh-c split in halves to
    shorten the critical chain), DVE forms c and h, PE transposes h back
    into the h^T ring (which doubles as the output staging buffer, DMA'd
    out once per 8 steps).
"""

import sys
if "/opt/trn_rl_repo" not in sys.path:
    sys.path.insert(0, "/opt/trn_rl_repo")
import numpy as np

import concourse.bass as bass
import concourse.bacc as bacc
import concourse.mybir as mybir
import concourse.tile as tile

F32 = mybir.dt.float32
F32R = mybir.dt.float32r
AF = mybir.ActivationFunctionType

S, B, I, H = 512, 64, 512, 512
NC = 8
BC = 16               # batch per core
G4 = 4 * H            # 2048 gate columns
NBLK = 8              # steps per token tile (128 tokens / BC)
N_TT = S // NBLK      # 64 token tiles
N_TOK = S * BC        # 8192 tokens per core


def _perm():
    # W rows are [i | f | g | o] (PyTorch LSTM gate order), 512 each.
    # Reorder columns of W.T to chunk order [g | i | f | o].
    idx = []
    for gate in (2, 0, 1, 3):
        idx.extend(range(gate * H, (gate + 1) * H))
    return np.array(idx, dtype=np.int64)


PERM = _perm()


def _slab(mat):
    """[512, N] -> [128, 4, N] with row 128k+p at [p, k, :]."""
    n = mat.shape[1]
    return np.ascontiguousarray(
        mat.reshape(4, 128, n).transpose(1, 0, 2), dtype=np.float32)


def prep_core_inputs(inpt, W_ih_f, W_hh_f, b_ih_f, b_hh_f,
                     W_ih_b, W_hh_b, b_ih_b, b_hh_b):
    x = {"f": np.ascontiguousarray(inpt),
         "b": np.ascontiguousarray(inpt[::-1])}
    wdat = {}
    for d, (Wih, Whh, bih, bhh) in (("f", (W_ih_f, W_hh_f, b_ih_f, b_hh_f)),
                                    ("b", (W_ih_b, W_hh_b, b_ih_b, b_hh_b))):
        wdat[d] = {
            "Wih": _slab(np.asarray(Wih, np.float32).T[:, PERM]),
            "Whh": _slab(np.asarray(Whh, np.float32).T[:, PERM]),
            "bias": np.ascontiguousarray(
                (np.asarray(bih) + np.asarray(bhh))[PERM],
                dtype=np.float32)[None, :],
        }

    selA = np.zeros((128, 32), dtype=np.float32)
    selB = np.zeros((128, 32), dtype=np.float32)
    for j in range(4):
        selA[32 * j:32 * j + 16, 0:16] = np.eye(16, dtype=np.float32)
        selB[32 * j + 16:32 * j + 32, 0:16] = np.eye(16, dtype=np.float32)
    eyeT = np.tile(np.eye(16, dtype=np.float32), (8, 1))
    ones1 = np.ones((1, 128), dtype=np.float32)

    in_maps = []
    for c in range(NC):
        d = "f" if c < 4 else "b"
        q = c % 4
        xs = x[d][:, q * BC:(q + 1) * BC, :]          # [S, 16, I]
        xT = xs.reshape(S * BC, I).T                   # [I, 8192]
        in_maps.append({
            "xT": _slab(xT),
            "Wih": wdat[d]["Wih"],
            "Whh": wdat[d]["Whh"],
            "bias": wdat[d]["bias"],
            "selA": selA, "selB": selB,
            "eyeT": eyeT, "ones1": ones1,
        })
    return in_maps


def assemble_output(results):
    out = np.empty((S, B, 2 * H), dtype=np.float32)
    for c in range(NC):
        q = c % 4
        off = 0 if c < 4 else H
        slab = results[c]["out"]                       # [64, 128, 8, 64]
        h = slab.reshape(N_TT, 128, NBLK, 4, BC)
        h = h.transpose(0, 2, 4, 3, 1).reshape(S, BC, H)
        out[:, q * BC:(q + 1) * BC, off:off + H] = h.astype(np.float32)
    return out


def build_nc(n_steps=S):
    nc = bacc.Bacc("TRN2", target_bir_lowering=False, debug=False)

    dram = {}
    dram["xT"] = nc.declare_dram_parameter("xT", [128, 4, N_TOK], F32R,
                                           isOutput=False)
    dram["Wih"] = nc.declare_dram_parameter("Wih", [128, 4, G4], F32R,
                                            isOutput=False)
    dram["Whh"] = nc.declare_dram_parameter("Whh", [128, 4, G4], F32R,
                                            isOutput=False)
    dram["bias"] = nc.declare_dram_parameter("bias", [1, G4], F32R,
                                             isOutput=False)
    dram["selA"] = nc.declare_dram_parameter("selA", [128, 32], F32R,
                                             isOutput=False)
    dram["selB"] = nc.declare_dram_parameter("selB", [128, 32], F32R,
                                             isOutput=False)
    dram["eyeT"] = nc.declare_dram_parameter("eyeT", [128, 16], F32R,
                                             isOutput=False)
    dram["ones1"] = nc.declare_dram_parameter("ones1", [1, 128], F32R,
                                              isOutput=False)
    n_tt = (n_steps + NBLK - 1) // NBLK
    dram["out"] = nc.declare_dram_parameter("out", [n_tt, 128, NBLK, 4 * BC],
                                            F32R, isOutput=True)
    xproj = nc.dram_tensor("xproj", [n_tt, 128, 4, 512], F32R)

    with tile.TileContext(nc) as tc:
        with (
            tc.tile_pool(name="weights", bufs=1) as wpool,
            tc.tile_pool(name="p1x", bufs=2) as p1x,
            tc.tile_pool(name="p1out", bufs=2) as p1out,
            tc.tile_pool(name="p1ps", bufs=1, space="PSUM") as p1ps,
            tc.tile_pool(name="xsp", bufs=3) as xsp,
            tc.tile_pool(name="gps", bufs=1, space="PSUM") as gps,
            tc.tile_pool(name="tps", bufs=1, space="PSUM") as tps,
            tc.tile_pool(name="ring", bufs=2) as rpool,
            tc.tile_pool(name="eltw", bufs=2) as epool,
        ):
            # ---- resident weights & constants ------------------------------
            Whh_sb = wpool.tile([128, 4, G4], F32R, tag="whh", name="whh")
            Wih_sb = wpool.tile([128, 4, G4], F32R, tag="wih", name="wih")
            bias_sb = wpool.tile([1, G4], F32R, tag="bias", name="bias_sb")
            selA_sb = wpool.tile([128, 32], F32R, tag="selA", name="selA_sb")
            selB_sb = wpool.tile([128, 32], F32R, tag="selB", name="selB_sb")
            eyeT_sb = wpool.tile([128, 16], F32R, tag="eyeT", name="eyeT_sb")
            ones1_sb = wpool.tile([1, 128], F32R, tag="ones1", name="ones1_sb")
            for k in range(4):
                nc.sync.dma_start(Whh_sb[:, k, :], dram["Whh"][:, k, :])
                nc.sync.dma_start(Wih_sb[:, k, :], dram["Wih"][:, k, :])
            nc.sync.dma_start(bias_sb[:, :], dram["bias"][:, :])
            nc.sync.dma_start(selA_sb[:, :], dram["selA"][:, :])
            nc.sync.dma_start(selB_sb[:, :], dram["selB"][:, :])
            nc.sync.dma_start(eyeT_sb[:, :], dram["eyeT"][:, :])
            nc.sync.dma_start(ones1_sb[:, :], dram["ones1"][:, :])

            hT0 = wpool.tile([128, 4 * BC], F32R, tag="hT0", name="hT0")
            nc.vector.memset(hT0[:, :].bitcast(F32), 0.0)

            # ---- phase 1 ---------------------------------------------------
            xt_tiles = {}

            def emit_p1_load(i):
                xt = p1x.tile([128, 4, 128], F32R, tag="xt", name=f"xt{i}")
                nc.sync.dma_start(xt[:, :, :],
                                  dram["xT"][:, :, i * 128:(i + 1) * 128])
                xt_tiles[i] = xt

            def emit_p1_chunk(i, cch):
                xt = xt_tiles[i]
                ps = p1ps.tile([128, 512], F32, tag="p1ps",
                               name=f"p1ps{i}_{cch}")
                for k in range(4):
                    nc.tensor.matmul(ps[:, :], xt[:, k, :],
                                     Wih_sb[:, k, cch * 512:(cch + 1) * 512],
                                     start=(k == 0), stop=False)
                nc.tensor.matmul(ps[:, :], ones1_sb[:, :],
                                 bias_sb[:, cch * 512:(cch + 1) * 512],
                                 start=False, stop=True)
                xo = p1out.tile([128, 512], F32R, tag="p1o",
                                name=f"p1o{i}_{cch}")
                nc.scalar.copy(xo[:, :], ps[:, :])
                nc.sync.dma_start(xproj[i, :, cch, :], xo[:, :])
                if cch == 3:
                    del xt_tiles[i]

            xs_tiles = {}

            def emit_xs_load(i):
                xs = xsp.tile([128, 4, 512], F32R, tag="xs", name=f"xs{i}")
                nc.sync.dma_start(xs[:, :, :], xproj[i, :, :, :])
                xs_tiles[i] = xs

            # startup: phase-1 for tiles 0..3, xs for tiles 0..1
            for i in range(min(4, n_tt)):
                emit_p1_load(i)
                for cch in range(4):
                    emit_p1_chunk(i, cch)
            for i in range(min(2, n_tt)):
                emit_xs_load(i)

            # ---- phase 2: recurrence ---------------------------------------
            ring_tiles = {}
            c_prev = epool.tile([BC, H], F32, tag="cst", name="c_init")
            nc.vector.memset(c_prev[:, :], 0.0)
            h_prev = None          # h tile of step t-1 (SBUF, [16, 512])
            GBUFS = (1, 1, 1, 2)   # PSUM bufs per gate chunk g,i,f,o

            def ring_ap(u):
                return ring_tiles[u // NBLK][:, u % NBLK, :]

            def hT_ap(u):
                # h^T slab [128, 64] of step u (-1 => zeros)
                return hT0[:, :] if u < 0 else ring_ap(u)

            def emit_tp(u):
                # transpose h(u) [16,512] -> ring slot u as h^T [128,64],
                # two PSUM banks (k01 / k23) so the first copies land early
                hu = h_prev
                rs = ring_ap(u)
                for half in range(2):
                    pt = tps.tile([128, 32], F32R, tag=f"pt{half}",
                                  name=f"pt{half}_{u}")
                    for kk in range(2):
                        k = 2 * half + kk
                        nc.tensor.matmul(
                            pt[:, kk * 16:(kk + 1) * 16],
                            hu[:, k * 128:(k + 1) * 128],
                            eyeT_sb[0:BC, :],
                            start=(kk == 0), stop=(kk == 1),
                            is_transpose=True, skip_group_check=True)
                    nc.vector.tensor_copy(rs[:, 32 * half:32 * half + 32],
                                          pt[:, :])

            for t in range(n_steps):
                blk, s = divmod(t, NBLK)
                if s == 0:
                    ring_tiles[blk] = rpool.tile(
                        [128, NBLK, 4 * BC], F32R, tag="ring",
                        name=f"ring{blk}")
                    if blk + 2 < n_tt:
                        emit_xs_load(blk + 2)
                    if blk + 4 < n_tt:
                        emit_p1_load(blk + 4)
                if s in (0, 2, 4, 6) and blk + 4 < n_tt:
                    emit_p1_chunk(blk + 4, s // 2)

                xs = xs_tiles[blk]
                sel = selA_sb if s % 2 == 0 else selB_sb
                q32 = 32 * (s // 2)

                # xproj injection (first matmul of each gate's group)
                gt = []
                for cch in range(4):
                    g = gps.tile([BC, 512], F32, tag=f"g{cch}",
                                 name=f"g{cch}_{t}", bufs=GBUFS[cch])
                    nc.tensor.matmul(
                        g[:, :], sel[q32:q32 + 32, 0:BC],
                        xs[q32:q32 + 32, cch, :],
                        start=True, stop=False, skip_group_check=True,
                        tile_position=(q32, 0))
                    gt.append(g)

                # previous step: transpose h into the ring, ship finished blk
                if t > 0:
                    emit_tp(t - 1)
                    if t % NBLK == 0:
                        pb = blk - 1
                        nc.sync.dma_start(dram["out"][pb, :, :, :],
                                          ring_tiles[pb][:, :, :])

                # recurrence matmuls: gates += h(t-1) @ W_hh
                hT = hT_ap(t - 1)
                for cch in range(4):
                    for k in range(4):
                        nc.tensor.matmul(
                            gt[cch][:, :], hT[:, k * BC:(k + 1) * BC],
                            Whh_sb[:, k, cch * 512:(cch + 1) * 512],
                            start=False, stop=(k == 3),
                            skip_group_check=True)

                # eltwise: c = sig(f)*c + sig(i)*tanh(g); h = sig(o)*tanh(c)
                tg = epool.tile([BC, H], F32, tag="tg", name=f"tg{t}")
                si = epool.tile([BC, H], F32, tag="si", name=f"si{t}")
                ig = epool.tile([BC, H], F32, tag="ig", name=f"ig{t}")
                sf = epool.tile([BC, H], F32, tag="sf", name=f"sf{t}")
                fc = epool.tile([BC, H], F32, tag="fc", name=f"fc{t}")
                so = epool.tile([BC, H], F32, tag="so", name=f"so{t}")
                tcc = epool.tile([BC, H], F32, tag="tcc", name=f"tcc{t}")
                c_new = epool.tile([BC, H], F32, tag="cst", name=f"cst{t}")
                h = epool.tile([BC, H], F32R, tag="h", name=f"h{t}")

                nc.scalar.activation(tg[:, :], gt[0][:, :], AF.Tanh)
                nc.scalar.activation(si[:, :], gt[1][:, :], AF.Sigmoid)
                nc.vector.tensor_mul(ig[:, :], si[:, :], tg[:, :])
                for hh in range(2):
                    cs = slice(hh * 256, (hh + 1) * 256)
                    nc.scalar.activation(sf[:, cs], gt[2][:, cs], AF.Sigmoid)
                    nc.vector.tensor_mul(fc[:, cs], sf[:, cs], c_prev[:, cs])
                    nc.vector.tensor_add(c_new[:, cs], ig[:, cs], fc[:, cs])
                    nc.scalar.activation(tcc[:, cs], c_new[:, cs], AF.Tanh)
                    nc.scalar.activation(so[:, cs], gt[3][:, cs], AF.Sigmoid)
                    nc.vector.tensor_mul(h[:, cs], so[:, cs], tcc[:, cs])

                c_prev = c_new
                h_prev = h

            emit_tp(n_steps - 1)
            lb = (n_steps - 1) // NBLK
            nc.sync.dma_start(dram["out"][lb, :, :, :],
                              ring_tiles[lb][:, :, :])

    nc.compile()
    return nc


# ---------------------------------------------------------------------------
# Entry point: kernel(**inputs) -> np.ndarray  [S, B, 2H]
# ---------------------------------------------------------------------------
from concourse.bass_utils import run_bass_kernel_spmd

_NC_CACHE = {}


def _get_nc():
    if "nc" not in _NC_CACHE:
        _NC_CACHE["nc"] = build_nc(n_steps=S)
    return _NC_CACHE["nc"]


def kernel(**inputs):
    nc = _get_nc()
    in_maps = prep_core_inputs(**inputs)
    res = run_bass_kernel_spmd(nc, in_maps, list(range(NC)))
    return assemble_output(res.results)


# revision 24
# speedup vs baseline: 15.8967x; 1.4151x over previous
"""Bass/Tile kernel for the bidirectional LSTM (S=512, B=64, I=H=512).

Sharding: direction-split + batch-split. Cores 0-3 run the FORWARD scan on
batch quarters of 16; cores 4-7 run the BACKWARD scan (input pre-flipped on
the host) on the same batch quarters. All cores execute the same SPMD
program; the direction lives entirely in the per-core weights/inputs.

Per core:
  Phase 1 (interleaved): xproj = x @ W_ih.T + b as a weights-stationary GEMM
    over 64 token tiles of 128 tokens, written to a DRAM ring
    xproj[tile, 128, 4, 512] (gate-chunk-major columns, PERM-reordered).
  Phase 2: 512-step recurrence, 8 steps per token tile.
    Gate columns reordered to chunks [g | i | f | o] (512 each).
    Per step: 4 selector matmuls inject xproj rows into the 4 gate PSUM
    banks (K=32 token window, eye(16) selector picks the step's 16 rows),
    then 4x4 W_hh matmuls (lhsT = h^T slab [128,16] per ktile) accumulate
    h @ W_hh. ACT applies tanh/sigmoid (f/o/tanh-c split in halves to
    shorten the critical chain), DVE forms c and h, PE transposes h back
    into the h^T ring (which doubles as the output staging buffer, DMA'd
    out once per 8 steps).
"""

import sys
if "/opt/trn_rl_repo" not in sys.path:
    sys.path.insert(0, "/opt/trn_rl_repo")
import numpy as np

import concourse.bass as bass
import concourse.bacc as bacc
import concourse.mybir as mybir
import concourse.tile as tile

F32 = mybir.dt.float32
F32R = mybir.dt.float32r
AF = mybir.ActivationFunctionType

S, B, I, H = 512, 64, 512, 512
NC = 8
BC = 16               # batch per core
G4 = 4 * H            # 2048 gate columns
NBLK = 8              # steps per token tile (128 tokens / BC)
N_TT = S // NBLK      # 64 token tiles
N_TOK = S * BC        # 8192 tokens per core


def _perm():
    # W rows are [i | f | g | o] (PyTorch LSTM gate order), 512 each.
    # Reorder columns of W.T to chunk order [g | i | f | o].
    idx = []
    for gate in (2, 0, 1, 3):
        idx.extend(range(gate * H, (gate + 1) * H))
    return np.array(idx, dtype=np.int64)


PERM = _perm()


def _slab(mat):
    """[512, N] -> [128, 4, N] with row 128k+p at [p, k, :]."""
    n = mat.shape[1]
    return np.ascontiguousarray(
        mat.reshape(4, 128, n).transpose(1, 0, 2), dtype=np.float32)


def prep_core_inputs(inpt, W_ih_f, W_hh_f, b_ih_f, b_hh_f,
                     W_ih_b, W_hh_b, b_ih_b, b_hh_b):
    x = {"f": np.ascontiguousarray(inpt),
         "b": np.ascontiguousarray(inpt[::-1])}
    wdat = {}
    for d, (Wih, Whh, bih, bhh) in (("f", (W_ih_f, W_hh_f, b_ih_f, b_hh_f)),
                                    ("b", (W_ih_b, W_hh_b, b_ih_b, b_hh_b))):
        wdat[d] = {
            "Wih": _slab(np.asarray(Wih, np.float32).T[:, PERM]),
            "Whh": _slab(np.asarray(Whh, np.float32).T[:, PERM]),
            "bias": np.ascontiguousarray(
                (np.asarray(bih) + np.asarray(bhh))[PERM],
                dtype=np.float32)[None, :],
        }

    selA = np.zeros((128, 32), dtype=np.float32)
    selB = np.zeros((128, 32), dtype=np.float32)
    for j in range(4):
        selA[32 * j:32 * j + 16, 0:16] = np.eye(16, dtype=np.float32)
        selB[32 * j + 16:32 * j + 32, 0:16] = np.eye(16, dtype=np.float32)
    eyeT = np.tile(np.eye(16, dtype=np.float32), (8, 1))
    ones1 = np.ones((1, 128), dtype=np.float32)

    in_maps = []
    for c in range(NC):
        d = "f" if c < 4 else "b"
        q = c % 4
        xs = x[d][:, q * BC:(q + 1) * BC, :]          # [S, 16, I]
        xT = xs.reshape(S * BC, I).T                   # [I, 8192]
        in_maps.append({
            "xT": _slab(xT),
            "Wih": wdat[d]["Wih"],
            "Whh": wdat[d]["Whh"],
            "bias": wdat[d]["bias"],
            "selA": selA, "selB": selB,
            "eyeT": eyeT, "ones1": ones1,
        })
    return in_maps


def assemble_output(results):
    out = np.empty((S, B, 2 * H), dtype=np.float32)
    for c in range(NC):
        q = c % 4
        off = 0 if c < 4 else H
        slab = results[c]["out"]                       # [64, 128, 8, 64]
        h = slab.reshape(N_TT, 128, NBLK, 4, BC)
        h = h.transpose(0, 2, 4, 3, 1).reshape(S, BC, H)
        out[:, q * BC:(q + 1) * BC, off:off + H] = h.astype(np.float32)
    return out


def build_nc(n_steps=S):
    nc = bacc.Bacc("TRN2", target_bir_lowering=False, debug=False)

    dram = {}
    dram["xT"] = nc.declare_dram_parameter("xT", [128, 4, N_TOK], F32R,
                                           isOutput=False)
    dram["Wih"] = nc.declare_dram_parameter("Wih", [128, 4, G4], F32R,
                                            isOutput=False)
    dram["Whh"] = nc.declare_dram_parameter("Whh", [128, 4, G4], F32R,
                                            isOutput=False)
    dram["bias"] = nc.declare_dram_parameter("bias", [1, G4], F32R,
                                             isOutput=False)
    dram["selA"] = nc.declare_dram_parameter("selA", [128, 32], F32R,
                                             isOutput=False)
    dram["selB"] = nc.declare_dram_parameter("selB", [128, 32], F32R,
                                             isOutput=False)
    dram["eyeT"] = nc.declare_dram_parameter("eyeT", [128, 16], F32R,
                                             isOutput=False)
    dram["ones1"] = nc.declare_dram_parameter("ones1", [1, 128], F32R,
                                              isOutput=False)
    n_tt = (n_steps + NBLK - 1) // NBLK
    dram["out"] = nc.declare_dram_parameter("out", [n_tt, 128, NBLK, 4 * BC],
                                            F32R, isOutput=True)
    xproj = nc.dram_tensor("xproj", [n_tt, 128, 4, 512], F32R)

    with tile.TileContext(nc) as tc:
        with (
            tc.tile_pool(name="weights", bufs=1) as wpool,
            tc.tile_pool(name="p1x", bufs=2) as p1x,
            tc.tile_pool(name="p1out", bufs=2) as p1out,
            tc.tile_pool(name="p1ps", bufs=1, space="PSUM") as p1ps,
            tc.tile_pool(name="xsp", bufs=3) as xsp,
            tc.tile_pool(name="gps", bufs=1, space="PSUM") as gps,
            tc.tile_pool(name="tps", bufs=1, space="PSUM") as tps,
            tc.tile_pool(name="ring", bufs=2) as rpool,
            tc.tile_pool(name="eltw", bufs=2) as epool,
        ):
            # ---- resident weights & constants ------------------------------
            Whh_sb = wpool.tile([128, 4, G4], F32R, tag="whh", name="whh")
            Wih_sb = wpool.tile([128, 4, G4], F32R, tag="wih", name="wih")
            bias_sb = wpool.tile([1, G4], F32R, tag="bias", name="bias_sb")
            selA_sb = wpool.tile([128, 32], F32R, tag="selA", name="selA_sb")
            selB_sb = wpool.tile([128, 32], F32R, tag="selB", name="selB_sb")
            eyeT_sb = wpool.tile([128, 16], F32R, tag="eyeT", name="eyeT_sb")
            ones1_sb = wpool.tile([1, 128], F32R, tag="ones1", name="ones1_sb")
            for k in range(4):
                nc.sync.dma_start(Whh_sb[:, k, :], dram["Whh"][:, k, :])
                nc.sync.dma_start(Wih_sb[:, k, :], dram["Wih"][:, k, :])
            nc.sync.dma_start(bias_sb[:, :], dram["bias"][:, :])
            nc.sync.dma_start(selA_sb[:, :], dram["selA"][:, :])
            nc.sync.dma_start(selB_sb[:, :], dram["selB"][:, :])
            nc.sync.dma_start(eyeT_sb[:, :], dram["eyeT"][:, :])
            nc.sync.dma_start(ones1_sb[:, :], dram["ones1"][:, :])

            hT0 = wpool.tile([128, 4 * BC], F32R, tag="hT0", name="hT0")
            nc.vector.memset(hT0[:, :].bitcast(F32), 0.0)

            # ---- phase 1 ---------------------------------------------------
            xt_tiles = {}

            def emit_p1_load(i):
                xt = p1x.tile([128, 4, 128], F32R, tag="xt", name=f"xt{i}")
                nc.sync.dma_start(xt[:, :, :],
                                  dram["xT"][:, :, i * 128:(i + 1) * 128])
                xt_tiles[i] = xt

            def emit_p1_chunk(i, cch):
                xt = xt_tiles[i]
                ps = p1ps.tile([128, 512], F32, tag="p1ps",
                               name=f"p1ps{i}_{cch}")
                for k in range(4):
                    nc.tensor.matmul(ps[:, :], xt[:, k, :],
                                     Wih_sb[:, k, cch * 512:(cch + 1) * 512],
                                     start=(k == 0), stop=False)
                nc.tensor.matmul(ps[:, :], ones1_sb[:, :],
                                 bias_sb[:, cch * 512:(cch + 1) * 512],
                                 start=False, stop=True)
                xo = p1out.tile([128, 512], F32R, tag="p1o",
                                name=f"p1o{i}_{cch}")
                nc.vector.tensor_copy(xo[:, :], ps[:, :])
                nc.sync.dma_start(xproj[i, :, cch, :], xo[:, :])
                if cch == 3:
                    del xt_tiles[i]

            xs_tiles = {}

            def emit_xs_load(i):
                xs = xsp.tile([128, 4, 512], F32R, tag="xs", name=f"xs{i}")
                nc.sync.dma_start(xs[:, :, :], xproj[i, :, :, :])
                xs_tiles[i] = xs

            # startup: phase-1 for tiles 0..3, xs for tiles 0..1
            for i in range(min(4, n_tt)):
                emit_p1_load(i)
                for cch in range(4):
                    emit_p1_chunk(i, cch)
            for i in range(min(2, n_tt)):
                emit_xs_load(i)

            # ---- phase 2: recurrence ---------------------------------------
            ring_tiles = {}
            c_prev = epool.tile([BC, H], F32, tag="cst", name="c_init")
            nc.vector.memset(c_prev[:, :], 0.0)
            h_prev = None          # h tile of step t-1 (SBUF, [16, 512])
            GBUFS = (1, 1, 1, 1)   # PSUM bufs per gate chunk g,i,f,o

            def ring_ap(u):
                return ring_tiles[u // NBLK][:, u % NBLK, :]

            def hT_ap(u):
                # h^T slab [128, 64] of step u (-1 => zeros)
                return hT0[:, :] if u < 0 else ring_ap(u)

            def emit_tp(u):
                # transpose h(u) [16,512] -> ring slot u as h^T [128,64],
                # two PSUM banks (k01 / k23) so the first copy lands early
                hu = h_prev
                rs = ring_ap(u)
                for half in range(2):
                    pt = tps.tile([128, 32], F32R, tag=f"pt{half}",
                                  name=f"pt{half}_{u}")
                    for kk in range(2):
                        k = 2 * half + kk
                        nc.tensor.matmul(
                            pt[:, kk * 16:(kk + 1) * 16],
                            hu[:, k * 128:(k + 1) * 128],
                            eyeT_sb[0:BC, :],
                            start=(kk == 0), stop=(kk == 1),
                            is_transpose=True, skip_group_check=True)
                    nc.vector.tensor_copy(rs[:, 32 * half:32 * half + 32],
                                          pt[:, :])

            for t in range(n_steps):
                blk, s = divmod(t, NBLK)
                if s == 0:
                    ring_tiles[blk] = rpool.tile(
                        [128, NBLK, 4 * BC], F32R, tag="ring",
                        name=f"ring{blk}")
                    if blk + 2 < n_tt:
                        emit_xs_load(blk + 2)
                    if blk + 4 < n_tt:
                        emit_p1_load(blk + 4)
                if s in (0, 2, 4, 6) and blk + 4 < n_tt:
                    emit_p1_chunk(blk + 4, s // 2)

                xs = xs_tiles[blk]
                sel = selA_sb if s % 2 == 0 else selB_sb
                q32 = 32 * (s // 2)

                def emit_sel(cch, cols=slice(0, 512), tag=None):
                    g = gps.tile([BC, cols.stop - cols.start], F32,
                                 tag=tag or f"g{cch}",
                                 name=f"{tag or f'g{cch}'}_{t}",
                                 bufs=GBUFS[cch])
                    nc.tensor.matmul(
                        g[:, :], sel[q32:q32 + 32, 0:BC],
                        xs[q32:q32 + 32, cch, cols],
                        start=True, stop=False, skip_group_check=True,
                        tile_position=(q32, 0))
                    return g

                def emit_hh(g, cch, cols=slice(0, 512)):
                    hT = hT_ap(t - 1)
                    for k in range(4):
                        nc.tensor.matmul(
                            g[:, :], hT[:, k * BC:(k + 1) * BC],
                            Whh_sb[:, k, cch * 512 + cols.start:
                                   cch * 512 + cols.stop],
                            start=False, stop=(k == 3),
                            skip_group_check=True)

                # g/i injections first (their banks freed early last step);
                # they give PE work to chew on during the t-1 tail.
                gt = [emit_sel(0), emit_sel(1)]

                # previous step: transpose h into the ring, ship finished blk
                if t > 0:
                    emit_tp(t - 1)
                    if t % NBLK == 0:
                        pb = blk - 1
                        nc.sync.dma_start(dram["out"][pb, :, :, :],
                                          ring_tiles[pb][:, :, :])

                # recurrence matmuls: gates += h(t-1) @ W_hh; f/o injections
                # sit just before their hh group so their bank-release wait
                # (late-tail ACTs of step t-1) never blocks the PE sequencer.
                # The o chunk is split into two half-width banks with their
                # own stops so sig(o) on cols 0:256 starts ~0.4us earlier.
                emit_hh(gt[0], 0)
                emit_hh(gt[1], 1)
                gt.append(emit_sel(2))
                emit_hh(gt[2], 2)
                oa = emit_sel(3, slice(0, 256), tag="goa")
                ob = emit_sel(3, slice(256, 512), tag="gob")
                emit_hh(oa, 3, slice(0, 256))
                emit_hh(ob, 3, slice(256, 512))
                gt.append((oa, ob))

                # eltwise: c = sig(f)*c + sig(i)*tanh(g); h = sig(o)*tanh(c)
                tg = epool.tile([BC, H], F32, tag="tg", name=f"tg{t}")
                si = epool.tile([BC, H], F32, tag="si", name=f"si{t}")
                ig = epool.tile([BC, H], F32, tag="ig", name=f"ig{t}")
                sf = epool.tile([BC, H], F32, tag="sf", name=f"sf{t}")
                fc = epool.tile([BC, H], F32, tag="fc", name=f"fc{t}")
                so = epool.tile([BC, H], F32, tag="so", name=f"so{t}")
                tcc = epool.tile([BC, H], F32, tag="tcc", name=f"tcc{t}")
                c_new = epool.tile([BC, H], F32, tag="cst", name=f"cst{t}")
                h = epool.tile([BC, H], F32R, tag="h", name=f"h{t}")

                nc.scalar.activation(tg[:, :], gt[0][:, :], AF.Tanh)
                nc.scalar.activation(si[:, :], gt[1][:, :], AF.Sigmoid)
                nc.vector.tensor_mul(ig[:, :], si[:, :], tg[:, :])
                ghalf = (gt[3][0][:, :], gt[3][1][:, :])
                H0, H1 = slice(0, 256), slice(256, 512)
                nc.scalar.activation(sf[:, H0], gt[2][:, H0], AF.Sigmoid)
                nc.scalar.activation(sf[:, H1], gt[2][:, H1], AF.Sigmoid)
                nc.vector.tensor_mul(fc[:, H0], sf[:, H0], c_prev[:, H0])
                nc.vector.tensor_add(c_new[:, H0], ig[:, H0], fc[:, H0])
                nc.vector.tensor_mul(fc[:, H1], sf[:, H1], c_prev[:, H1])
                nc.vector.tensor_add(c_new[:, H1], ig[:, H1], fc[:, H1])
                nc.scalar.activation(so[:, H0], ghalf[0], AF.Sigmoid)
                nc.scalar.activation(so[:, H1], ghalf[1], AF.Sigmoid)
                nc.scalar.activation(tcc[:, H0], c_new[:, H0], AF.Tanh)
                nc.scalar.activation(tcc[:, H1], c_new[:, H1], AF.Tanh)
                nc.vector.tensor_mul(h[:, H0], so[:, H0], tcc[:, H0])
                nc.vector.tensor_mul(h[:, H1], so[:, H1], tcc[:, H1])

                c_prev = c_new
                h_prev = h

            emit_tp(n_steps - 1)
            lb = (n_steps - 1) // NBLK
            nc.sync.dma_start(dram["out"][lb, :, :, :],
                              ring_tiles[lb][:, :, :])

    nc.compile()
    return nc


# ---------------------------------------------------------------------------
# Entry point: kernel(**inputs) -> np.ndarray  [S, B, 2H]
# ---------------------------------------------------------------------------
from concourse.bass_utils import run_bass_kernel_spmd

_NC_CACHE = {}


def _get_nc():
    if "nc" not in _NC_CACHE:
        _NC_CACHE["nc"] = build_nc(n_steps=S)
    return _NC_CACHE["nc"]


def kernel(**inputs):
    nc = _get_nc()
    in_maps = prep_core_inputs(**inputs)
    res = run_bass_kernel_spmd(nc, in_maps, list(range(NC)))
    return assemble_output(res.results)
